# revision 1
# baseline (speedup 1.0000x reference)
"""Trainium2 Bass kernel for nn_KeyedConv2d: 3x3 SAME conv, stride 1.

x: [8, 64, 64, 64] (NCHW), Wt: [64, 64, 3, 3] (OIHW) -> out [8, 64, 64, 64].

Sharding: data-parallel over batch, one image per NeuronCore (8 cores).

Per-core algorithm: conv = sum over the 9 kernel offsets of a [IC=64 x OC=64]
matmul applied to a shifted view of the zero-padded image held in SBUF.
The padded image [64, 66*66] is duplicated into both SBUF partition halves so
two output chunks (512 pixels each) run concurrently on the two 64-row strips
of the PE array (tile_position row packing; fp32r forbids column packing).
Each strip accumulates its chunk's 9 offsets into its own PSUM bank; DVE
copies PSUM->SBUF and DMA stores to HBM.  Matmuls run in float32r (full PE
rate; ~1e-4 scaled error) -- set MODE="f32" for exact-but-4x-slower matmuls.
"""
import numpy as np

import concourse.bass as bass
import concourse.mybir as mybir
import concourse.tile as tile
from concourse import bacc
from concourse.bass_utils import run_bass_kernel_spmd

F32 = mybir.dt.float32
F32R = mybir.dt.float32r

IC = OC = 64
H = W = 64
K = 3
PH = H + 2          # vertically padded height 66
PW = W + 1          # one shared zero column per row (left pad; also serves
                    # as the right pad of the previous row when a kx=2 view
                    # reads contiguously across the row boundary)
PSZ = PW * PH       # 4290
ALLOC = PSZ + 14    # slack so the last kx=2 view's 520-elem slice stays in range
HWPIX = H * W       # 4096
CHUNK = 512         # output pixels per matmul (one PSUM bank)
NCH = HWPIX // CHUNK  # 8 chunks -> 4 chunk-pairs
RPC = CHUNK // W    # 8 image rows per chunk

OFFS = [(ky, kx) for ky in range(K) for kx in range(K)]

MODE = "f32r"       # "f32r" | "f32"


def _build(mode: str = MODE) -> bacc.Bacc:
    mm_dt = F32R if mode == "f32r" else F32
    nc = bacc.Bacc("TRN2", target_bir_lowering=False, debug=False)

    x = nc.dram_tensor("x", [IC, H, W], F32, kind="ExternalInput").ap()
    # host-pretransposed weights: wt[ic, (ky*3+kx)*64 + oc] = Wt[oc, ic, ky, kx]
    wt = nc.dram_tensor("wt", [IC, K * K * OC], F32, kind="ExternalInput").ap()
    zeros = nc.dram_tensor("zeros", [128, 96], F32, kind="ExternalInput").ap()
    y = nc.dram_tensor("y", [OC, HWPIX], F32, kind="ExternalOutput").ap()

    x_src = x.rearrange("c h w -> c (h w)")

    with tile.TileContext(nc) as tc:
        with (
            tc.tile_pool(name="xpad", bufs=1) as xpad_pool,
            tc.tile_pool(name="wsb", bufs=1) as wsb_pool,
            tc.tile_pool(name="osb", bufs=3) as osb_pool,
            tc.tile_pool(name="psum", bufs=4, space="PSUM") as psum_pool,
        ):
            # --- weights: [128, 576]; both halves hold the same data so
            # lhsT.base_partition matches the row strip.
            wsb = wsb_pool.tile([128, K * K * OC], mm_dt)
            for s in (0, 1):
                nc.sync.dma_start(wsb[64 * s:64 * s + 64, :], wt.bitcast(mm_dt))

            # --- padded image (65-wide rows) duplicated into both halves.
            xpad = xpad_pool.tile([128, ALLOC], mm_dt)
            xr = xpad[:, :PSZ].rearrange("p (a b) -> p a b", b=PW)
            zsrc = zeros.bitcast(mm_dt)
            # zero: top pad row, bottom pad row + slack, shared pad column
            nc.sync.dma_start(xpad[:, 0:PW], zsrc[:, :PW])
            nc.sync.dma_start(xpad[:, (PH - 1) * PW:], zsrc[:, :PW + 14])
            nc.sync.dma_start(
                xr[:, 1:PH - 1, 0:1],
                zsrc[:, :H].rearrange("p (a b) -> p a b", b=1),
            )
            # image rows -> rows 1..64, cols 1..64 (one DMA per half,
            # on different HWDGE engines so the queues run in parallel)
            for s in (0, 1):
                nc.sync.dma_start(
                    xr[64 * s:64 * s + 64, 1:PH - 1, 1:PW],
                    x_src.bitcast(mm_dt),
                )

            # --- conv: 4 chunk-pairs; row strip s handles chunk 2q+s with
            # all 9 offsets accumulating into its own PSUM bank.
            for q in range(NCH // 2):
                ps = [
                    psum_pool.tile([64, CHUNK], F32, name=f"ps{s}")
                    for s in (0, 1)
                ]
                for t, (ky, kx) in enumerate(OFFS):
                    for s in (0, 1):
                        c = 2 * q + s
                        o = (c * RPC + ky) * PW + kx
                        rhs = xpad[64 * s:64 * s + 64,
                                   o:o + RPC * PW].rearrange(
                            "p (a b) -> p a b", b=PW)[:, :, :W]
                        lhsT = wsb[64 * s:64 * s + 64,
                                   (ky * K + kx) * OC:(ky * K + kx + 1) * OC]
                        nc.tensor.matmul(
                            ps[s][:, :],
                            lhsT,
                            rhs,
                            start=(t == 0),
                            stop=(t == len(OFFS) - 1),
                            skip_group_check=True,
                        )

                # PSUM -> SBUF -> HBM (both chunks in one 256KB store)
                osb = osb_pool.tile([64, 2 * CHUNK], F32, name="osb")
                for s in (0, 1):
                    nc.vector.tensor_copy(
                        osb[:, s * CHUNK:(s + 1) * CHUNK], ps[s][:, :]
                    )
                nc.sync.dma_start(
                    y[:, 2 * q * CHUNK:(2 * q + 2) * CHUNK], osb[:, :]
                )

    nc.compile()
    return nc


_NC_CACHE: dict[str, bacc.Bacc] = {}
_ZEROS = np.zeros((128, 96), dtype=np.float32)


def kernel(x: np.ndarray, Wt: np.ndarray) -> np.ndarray:
    assert x.shape == (8, IC, H, W) and Wt.shape == (OC, IC, K, K)
    if MODE not in _NC_CACHE:
        _NC_CACHE[MODE] = _build(MODE)
    nc = _NC_CACHE[MODE]

    # wt[ic, (ky*3+kx)*64 + oc]
    wt_t = np.ascontiguousarray(
        Wt.astype(np.float32).transpose(1, 2, 3, 0).reshape(IC, K * K * OC)
    )
    in_maps = [
        {
            "x": np.ascontiguousarray(x[b], dtype=np.float32),
            "wt": wt_t,
            "zeros": _ZEROS,
        }
        for b in range(8)
    ]
    global _last_in_maps
    _last_in_maps = in_maps
    res = run_bass_kernel_spmd(nc, in_maps, core_ids=list(range(8)))
    out = np.stack([r["y"].reshape(OC, H, W) for r in res.results])
    return out.astype(np.float32)


_last_in_maps: list[dict[str, np.ndarray]] = []



# revision 5
# speedup vs baseline: 1.6234x; 1.6234x over previous
"""Trainium2 Bass kernel for nn_KeyedConv2d: 3x3 SAME conv, stride 1.

x: [8, 64, 64, 64] (NCHW), Wt: [64, 64, 3, 3] (OIHW) -> out [8, 64, 64, 64].

Sharding: data-parallel over batch, one image per NeuronCore (8 cores).

Per-core algorithm (v2):
  * x is DMAed contiguously (full-bandwidth 16KB descriptors) into xlin
    [128, 4096]; both partition halves hold the same image.
  * Pool (H0) / Activation (H1) engines re-layout each half into the padded
    image xpad [128, 66*65]: 65-wide rows with a shared zero pad column.
    H1 is placed one image row lower than H0, so a single contraction-128
    matmul reads (ky=0, ky=1) simultaneously: H0 partitions supply the
    ky=0 shifted view and H1 partitions the ky=1 view of the same columns.
  * Per 512-pixel chunk (8 image rows): 3 fused matmuls (ky=0+1, kx=0..2,
    contraction 128) + 3 single matmuls (ky=2, kx=0..2, contraction 64)
    accumulate into that chunk's PSUM bank.  48 matmuls total (vs 72 for
    the naive one-offset-per-matmul schedule).
  * Matmuls are gated chunk-by-chunk on the staged pad-copies, which keeps
    the tensor engine's dispatch pipeline saturated (full-rate f32r).
  * DVE copies PSUM->SBUF per chunk pair; merged [64, 1024] stores to HBM.
"""
import numpy as np

import concourse.bass as bass
import concourse.mybir as mybir
import concourse.tile as tile
from concourse import bacc
from concourse.bass_utils import run_bass_kernel_spmd

F32 = mybir.dt.float32
F32R = mybir.dt.float32r

IC = OC = 64
H = W = 64
K = 3
PW = W + 1          # 65: one shared zero column per padded row
PH = H + 2          # 66 padded rows
PSZ = PW * PH       # 4290
ALLOC = PSZ + 16    # slack so the last ky=2 view stays in range
HWPIX = H * W       # 4096
CHUNK = 512         # output pixels per PSUM bank (8 image rows)
NCH = HWPIX // CHUNK  # 8 chunks
RPC = CHUNK // W    # 8 image rows per chunk
NPIECE = 4          # x DMA pieces per half (16 image rows each)
PROWS = H // NPIECE  # 16
PCOLS = PROWS * W   # 1024 elements per piece per partition


def _build() -> bacc.Bacc:
    nc = bacc.Bacc("TRN2", target_bir_lowering=False, debug=False)

    x = nc.dram_tensor("x", [IC, HWPIX], F32, kind="ExternalInput").ap()
    # host-prepped weights [128, 384]:
    #   cols 0:192   fused pairs: [ic, kx*64+oc] = Wt[oc,ic,0,kx] (top half)
    #                             [64+ic, ...]   = Wt[oc,ic,1,kx] (bottom)
    #   cols 192:384 singles:     [ic, 192+kx*64+oc] = Wt[oc,ic,2,kx]
    wt = nc.dram_tensor("wt", [128, 2 * K * OC], F32, kind="ExternalInput").ap()
    y = nc.dram_tensor("y", [OC, HWPIX], F32, kind="ExternalOutput").ap()

    with tile.TileContext(nc) as tc:
        with (
            tc.tile_pool(name="wsb", bufs=1) as wsb_pool,
            tc.tile_pool(name="xlin", bufs=1) as xlin_pool,
            tc.tile_pool(name="xpad", bufs=1) as xpad_pool,
            tc.tile_pool(name="osb", bufs=2) as osb_pool,
            tc.tile_pool(name="psum", bufs=8, space="PSUM") as psum_pool,
        ):
            wsb = wsb_pool.tile([128, 2 * K * OC], F32R)
            xlin = xlin_pool.tile([128, HWPIX], F32R)
            xpad = xpad_pool.tile([128, ALLOC], F32R)
            xr = xpad[:, :PSZ].rearrange("p (a b) -> p a b", b=PW)

            # --- zero pads: engine memsets can't write f32r (ISA check), so
            # memset an F32 scratch and cast-copy (rounds to f32r). ---
            zs = xlin_pool.tile([128, 96], F32, name="zs")
            nc.vector.memset(zs[:, :], 0.0)
            # H0 top padded row 0 (incl col 0)
            nc.vector.tensor_copy(xpad[0:IC, 0:PW], zs[0:IC, 0:PW])
            # H0 bottom padded row 65 + slack (ky=2 view tail)
            nc.vector.tensor_copy(
                xpad[0:IC, (PH - 1) * PW:ALLOC],
                zs[0:IC, 0:ALLOC - (PH - 1) * PW])
            # shared zero column 0 of every padded row, both halves
            nc.vector.tensor_copy(
                xr[:, :, 0:1],
                zs[:, 0:PH].rearrange("p (a b) -> p a b", b=1))

            # --- weights ---
            nc.sync.dma_start(wsb, wt.bitcast(F32R))

            # --- x pieces: H0 then H1 per piece, interleaved so the padded
            # halves become ready in lockstep (chunk-wise matmul gating) ---
            for j in range(NPIECE):
                cs = slice(j * PCOLS, (j + 1) * PCOLS)
                nc.sync.dma_start(xlin[0:IC, cs], x.bitcast(F32R)[:, cs])
                nc.sync.dma_start(xlin[IC:128, cs], x.bitcast(F32R)[:, cs])

            # --- pad-copies: piece j covers image rows 16j..16j+15 ---
            # H0: image row r -> padded row r+1 (Pool engine)
            # H1: image row r -> padded row r   (Activation engine)
            for j in range(NPIECE):
                src = xlin[:, j * PCOLS:(j + 1) * PCOLS].rearrange(
                    "p (a b) -> p a b", b=W)
                r0 = j * PROWS
                nc.gpsimd.tensor_copy(
                    xr[0:IC, 1 + r0:1 + r0 + PROWS, 1:PW], src[0:IC])
                nc.scalar.copy(
                    xr[IC:128, r0:r0 + PROWS, 1:PW], src[IC:128])

            # --- conv: chunk c accumulates its 6 matmuls into PSUM bank c ---
            wr = wsb
            xrr = xpad
            pss = []
            for c in range(NCH):
                ps = psum_pool.tile([OC, CHUNK], F32, name="ps")
                pss.append(ps)
                # fused ky=0+1 (contraction 128)
                for kx in range(K):
                    o = (c * RPC) * PW + kx
                    rhs = xrr[:, o:o + RPC * PW].rearrange(
                        "p (a b) -> p a b", b=PW)[:, :, :W]
                    nc.tensor.matmul(
                        ps[:, :], wr[:, kx * OC:(kx + 1) * OC], rhs,
                        start=(kx == 0), stop=False, skip_group_check=True)
                # ky=2 singles (contraction 64, H0 only)
                for kx in range(K):
                    o = (c * RPC + 2) * PW + kx
                    rhs = xrr[0:IC, o:o + RPC * PW].rearrange(
                        "p (a b) -> p a b", b=PW)[:, :, :W]
                    nc.tensor.matmul(
                        ps[:, :], wr[0:IC, (K + kx) * OC:(K + kx + 1) * OC],
                        rhs, start=False, stop=(kx == K - 1),
                        skip_group_check=True)

                # drain per chunk pair
                if c % 2 == 1:
                    osb = osb_pool.tile([OC, 2 * CHUNK], F32, name="osb")
                    nc.vector.tensor_copy(osb[:, 0:CHUNK], pss[c - 1][:, :])
                    nc.vector.tensor_copy(osb[:, CHUNK:2 * CHUNK], ps[:, :])
                    nc.sync.dma_start(
                        y[:, (c - 1) * CHUNK:(c + 1) * CHUNK], osb[:, :])

    nc.compile()
    return nc


_NC_CACHE: dict[str, bacc.Bacc] = {}
MODE = "v2"


def kernel(x: np.ndarray, Wt: np.ndarray) -> np.ndarray:
    assert x.shape == (8, IC, H, W) and Wt.shape == (OC, IC, K, K)
    if MODE not in _NC_CACHE:
        _NC_CACHE[MODE] = _build()
    nc = _NC_CACHE[MODE]

    Wf = Wt.astype(np.float32)
    wt_t = np.zeros((128, 2 * K * OC), dtype=np.float32)
    # [O,I,kx] -> [I,kx,O] -> [64, 192]
    wt_t[0:IC, 0:192] = Wf[:, :, 0, :].transpose(1, 2, 0).reshape(IC, 192)
    wt_t[IC:128, 0:192] = Wf[:, :, 1, :].transpose(1, 2, 0).reshape(IC, 192)
    wt_t[0:IC, 192:384] = Wf[:, :, 2, :].transpose(1, 2, 0).reshape(IC, 192)

    in_maps = [
        {
            "x": np.ascontiguousarray(
                x[b].reshape(IC, HWPIX), dtype=np.float32),
            "wt": wt_t,
        }
        for b in range(8)
    ]
    global _last_in_maps
    _last_in_maps = in_maps
    res = run_bass_kernel_spmd(nc, in_maps, core_ids=list(range(8)))
    out = np.stack([r["y"].reshape(OC, H, W) for r in res.results])
    return out.astype(np.float32)


_last_in_maps: list[dict[str, np.ndarray]] = []


# revision 7
# speedup vs baseline: 2.1239x; 1.3083x over previous
"""Trainium2 Bass kernel for nn_KeyedConv2d: 3x3 SAME conv, stride 1.

x: [8, 64, 64, 64] (NCHW), Wt: [64, 64, 3, 3] (OIHW) -> out [8, 64, 64, 64].

Sharding: data-parallel over batch, one image per NeuronCore (8 cores).

Per-core algorithm (v2):
  * x is DMAed contiguously (full-bandwidth 16KB descriptors) into xlin
    [128, 4096]; both partition halves hold the same image.
  * Pool (H0) / Activation (H1) engines re-layout each half into the padded
    image xpad [128, 66*65]: 65-wide rows with a shared zero pad column.
    H1 is placed one image row lower than H0, so a single contraction-128
    matmul reads (ky=0, ky=1) simultaneously: H0 partitions supply the
    ky=0 shifted view and H1 partitions the ky=1 view of the same columns.
  * Per 512-pixel chunk (8 image rows): 3 fused matmuls (ky=0+1, kx=0..2,
    contraction 128) + 3 single matmuls (ky=2, kx=0..2, contraction 64)
    accumulate into that chunk's PSUM bank.  48 matmuls total (vs 72 for
    the naive one-offset-per-matmul schedule).
  * Matmuls are gated chunk-by-chunk on the staged pad-copies, which keeps
    the tensor engine's dispatch pipeline saturated (full-rate f32r).
  * DVE copies PSUM->SBUF per chunk pair; merged [64, 1024] stores to HBM.
"""
import numpy as np

import concourse.bass as bass
import concourse.mybir as mybir
import concourse.tile as tile
from concourse import bacc
from concourse.bass_utils import run_bass_kernel_spmd

F32 = mybir.dt.float32
F32R = mybir.dt.float32r

IC = OC = 64
H = W = 64
K = 3
PW = W + 1          # 65: one shared zero column per padded row
PH = H + 2          # 66 padded rows
PSZ = PW * PH       # 4290
ALLOC = PSZ + 16    # slack so the last ky=2 view stays in range
HWPIX = H * W       # 4096
CHUNK = 512         # output pixels per PSUM bank (8 image rows)
NCH = HWPIX // CHUNK  # 8 chunks
RPC = CHUNK // W    # 8 image rows per chunk
NPIECE = 4          # x DMA pieces per half (16 image rows each)
PROWS = H // NPIECE  # 16
PCOLS = PROWS * W   # 1024 elements per piece per partition


def _build() -> bacc.Bacc:
    nc = bacc.Bacc("TRN2", target_bir_lowering=False, debug=False)

    x = nc.dram_tensor("x", [IC, HWPIX], F32, kind="ExternalInput").ap()
    # host-prepped weights [128, 384]:
    #   cols 0:192   fused pairs: [ic, kx*64+oc] = Wt[oc,ic,0,kx] (top half)
    #                             [64+ic, ...]   = Wt[oc,ic,1,kx] (bottom)
    #   cols 192:384 singles:     [ic, 192+kx*64+oc] = Wt[oc,ic,2,kx]
    wt = nc.dram_tensor("wt", [128, 2 * K * OC], F32, kind="ExternalInput").ap()
    y = nc.dram_tensor("y", [OC, HWPIX], F32, kind="ExternalOutput").ap()

    with tile.TileContext(nc) as tc:
        with (
            tc.tile_pool(name="wsb", bufs=1) as wsb_pool,
            tc.tile_pool(name="xlin", bufs=1) as xlin_pool,
            tc.tile_pool(name="xpad", bufs=1) as xpad_pool,
            tc.tile_pool(name="osb", bufs=2) as osb_pool,
            tc.tile_pool(name="psum", bufs=8, space="PSUM") as psum_pool,
        ):
            wsb = wsb_pool.tile([128, 2 * K * OC], F32R)
            xlin = xlin_pool.tile([128, HWPIX], F32R)
            xpad = xpad_pool.tile([128, ALLOC], F32R)
            xr = xpad[:, :PSZ].rearrange("p (a b) -> p a b", b=PW)

            # --- zero pads: engine memsets can't write f32r (ISA check), so
            # memset an F32 scratch and cast-copy (rounds to f32r). ---
            zs = xlin_pool.tile([128, 96], F32, name="zs")
            nc.vector.memset(zs[:, :], 0.0)
            # H0 top padded row 0 (incl col 0)
            nc.vector.tensor_copy(xpad[0:IC, 0:PW], zs[0:IC, 0:PW])
            # H0 bottom padded row 65 + slack (ky=2 view tail)
            nc.vector.tensor_copy(
                xpad[0:IC, (PH - 1) * PW:ALLOC],
                zs[0:IC, 0:ALLOC - (PH - 1) * PW])
            # shared zero column 0 of every padded row, both halves
            nc.vector.tensor_copy(
                xr[:, :, 0:1],
                zs[:, 0:PH].rearrange("p (a b) -> p a b", b=1))

            # --- weights ---
            nc.sync.dma_start(wsb, wt.bitcast(F32R))

            # --- x pieces: small leading pieces for an early matmul start,
            # interleaved H0/H1 so the halves become ready in lockstep ---
            PIECES = [(0, 8), (8, 8), (16, 16), (32, 16), (48, 16)]
            for r0, nr in PIECES:
                cs = slice(r0 * W, (r0 + nr) * W)
                nc.sync.dma_start(xlin[0:IC, cs], x.bitcast(F32R)[:, cs])
                nc.sync.dma_start(xlin[IC:128, cs], x.bitcast(F32R)[:, cs])

            # --- pad-copies in 8-row sub-pieces (= 1 output chunk): each
            # completion unlocks just a few matmuls, which keeps the PE
            # dispatch queue shallow (cost model rewards this with the
            # full-rate p-state).
            # H0: image row r -> padded row r+1 (Pool engine)
            # H1: image row r -> padded row r   (Activation engine)
            for j in range(H // RPC):
                src = xlin[:, j * RPC * W:(j + 1) * RPC * W].rearrange(
                    "p (a b) -> p a b", b=W)
                r0 = j * RPC
                nc.gpsimd.tensor_copy(
                    xr[0:IC, 1 + r0:1 + r0 + RPC, 1:PW], src[0:IC])
                nc.scalar.copy(
                    xr[IC:128, r0:r0 + RPC, 1:PW], src[IC:128])

            # --- conv: chunk c accumulates its 6 matmuls into PSUM bank c ---
            wr = wsb
            xrr = xpad
            pss = []
            for c in range(NCH):
                ps = psum_pool.tile([OC, CHUNK], F32, name="ps")
                pss.append(ps)
                # fused ky=0+1 (contraction 128)
                for kx in range(K):
                    o = (c * RPC) * PW + kx
                    rhs = xrr[:, o:o + RPC * PW].rearrange(
                        "p (a b) -> p a b", b=PW)[:, :, :W]
                    nc.tensor.matmul(
                        ps[:, :], wr[:, kx * OC:(kx + 1) * OC], rhs,
                        start=(kx == 0), stop=False, skip_group_check=True)
                # ky=2 singles (contraction 64, H0 only)
                for kx in range(K):
                    o = (c * RPC + 2) * PW + kx
                    rhs = xrr[0:IC, o:o + RPC * PW].rearrange(
                        "p (a b) -> p a b", b=PW)[:, :, :W]
                    nc.tensor.matmul(
                        ps[:, :], wr[0:IC, (K + kx) * OC:(K + kx + 1) * OC],
                        rhs, start=False, stop=(kx == K - 1),
                        skip_group_check=True)

                # drain: chunk pairs early on, single chunks at the end
                # (shorter critical-path tail after the last matmul)
                if c in (1, 3, 5):
                    osb = osb_pool.tile([OC, 2 * CHUNK], F32, name="osb")
                    nc.vector.tensor_copy(osb[:, 0:CHUNK], pss[c - 1][:, :])
                    nc.vector.tensor_copy(osb[:, CHUNK:2 * CHUNK], ps[:, :])
                    nc.sync.dma_start(
                        y[:, (c - 1) * CHUNK:(c + 1) * CHUNK], osb[:, :])
                elif c in (6, 7):
                    osbs = osb_pool.tile([OC, CHUNK], F32, name="osbs")
                    nc.vector.tensor_copy(osbs[:, :], ps[:, :])
                    nc.sync.dma_start(
                        y[:, c * CHUNK:(c + 1) * CHUNK], osbs[:, :])

    nc.compile()
    return nc


_NC_CACHE: dict[str, bacc.Bacc] = {}
MODE = "v2"


def kernel(x: np.ndarray, Wt: np.ndarray) -> np.ndarray:
    assert x.shape == (8, IC, H, W) and Wt.shape == (OC, IC, K, K)
    if MODE not in _NC_CACHE:
        _NC_CACHE[MODE] = _build()
    nc = _NC_CACHE[MODE]

    Wf = Wt.astype(np.float32)
    wt_t = np.zeros((128, 2 * K * OC), dtype=np.float32)
    # [O,I,kx] -> [I,kx,O] -> [64, 192]
    wt_t[0:IC, 0:192] = Wf[:, :, 0, :].transpose(1, 2, 0).reshape(IC, 192)
    wt_t[IC:128, 0:192] = Wf[:, :, 1, :].transpose(1, 2, 0).reshape(IC, 192)
    wt_t[0:IC, 192:384] = Wf[:, :, 2, :].transpose(1, 2, 0).reshape(IC, 192)

    in_maps = [
        {
            "x": np.ascontiguousarray(
                x[b].reshape(IC, HWPIX), dtype=np.float32),
            "wt": wt_t,
        }
        for b in range(8)
    ]
    global _last_in_maps
    _last_in_maps = in_maps
    res = run_bass_kernel_spmd(nc, in_maps, core_ids=list(range(8)))
    out = np.stack([r["y"].reshape(OC, H, W) for r in res.results])
    return out.astype(np.float32)


_last_in_maps: list[dict[str, np.ndarray]] = []


# revision 9
# speedup vs baseline: 2.7654x; 1.3020x over previous
"""Trainium2 Bass kernel for nn_KeyedConv2d: 3x3 SAME conv, stride 1.

x: [8, 64, 64, 64] (NCHW), Wt: [64, 64, 3, 3] (OIHW) -> out [8, 64, 64, 64].

Sharding: data-parallel over batch, one image per NeuronCore (8 cores).

Per-core algorithm (v2):
  * x is DMAed contiguously (full-bandwidth 16KB descriptors) into xlin
    [128, 4096]; both partition halves hold the same image.
  * Pool (H0) / Activation (H1) engines re-layout each half into the padded
    image xpad [128, 66*65]: 65-wide rows with a shared zero pad column.
    H1 is placed one image row lower than H0, so a single contraction-128
    matmul reads (ky=0, ky=1) simultaneously: H0 partitions supply the
    ky=0 shifted view and H1 partitions the ky=1 view of the same columns.
  * Per 512-pixel chunk (8 image rows): 3 fused matmuls (ky=0+1, kx=0..2,
    contraction 128) + 3 single matmuls (ky=2, kx=0..2, contraction 64)
    accumulate into that chunk's PSUM bank.  48 matmuls total (vs 72 for
    the naive one-offset-per-matmul schedule).
  * Matmuls are gated chunk-by-chunk on the staged pad-copies, which keeps
    the tensor engine's dispatch pipeline saturated (full-rate f32r).
  * DVE copies PSUM->SBUF per chunk pair; merged [64, 1024] stores to HBM.
"""
import numpy as np

import concourse.bass as bass
import concourse.mybir as mybir
import concourse.tile as tile
from concourse import bacc
from concourse.bass_utils import run_bass_kernel_spmd

F32 = mybir.dt.float32
F32R = mybir.dt.float32r

IC = OC = 64
H = W = 64
K = 3
PW = W + 1          # 65: one shared zero column per padded row
PH = H + 2          # 66 padded rows
PSZ = PW * PH       # 4290
ALLOC = PSZ + 16    # slack so the last ky=2 view stays in range
HWPIX = H * W       # 4096
CHUNK = 512         # output pixels per PSUM bank (8 image rows)
NCH = HWPIX // CHUNK  # 8 chunks
RPC = CHUNK // W    # 8 image rows per chunk
NPIECE = 4          # x DMA pieces per half (16 image rows each)
PROWS = H // NPIECE  # 16
PCOLS = PROWS * W   # 1024 elements per piece per partition


def _build() -> bacc.Bacc:
    nc = bacc.Bacc("TRN2", target_bir_lowering=False, debug=False)

    x = nc.dram_tensor("x", [IC, HWPIX], F32, kind="ExternalInput").ap()
    # host-prepped weights [128, 384]:
    #   cols 0:192   fused pairs: [ic, kx*64+oc] = Wt[oc,ic,0,kx] (top half)
    #                             [64+ic, ...]   = Wt[oc,ic,1,kx] (bottom)
    #   cols 192:384 singles:     [ic, 192+kx*64+oc] = Wt[oc,ic,2,kx]
    wt = nc.dram_tensor("wt", [128, 2 * K * OC], F32, kind="ExternalInput").ap()
    y = nc.dram_tensor("y", [OC, HWPIX], F32, kind="ExternalOutput").ap()

    with tile.TileContext(nc) as tc:
        with (
            tc.tile_pool(name="wsb", bufs=1) as wsb_pool,
            tc.tile_pool(name="xlin", bufs=1) as xlin_pool,
            tc.tile_pool(name="xpad", bufs=1) as xpad_pool,
            tc.tile_pool(name="osb", bufs=2) as osb_pool,
            tc.tile_pool(name="psum", bufs=8, space="PSUM") as psum_pool,
        ):
            wsb = wsb_pool.tile([128, 2 * K * OC], F32R)
            xlin = xlin_pool.tile([128, HWPIX], F32R)
            xpad = xpad_pool.tile([128, ALLOC], F32R)
            xr = xpad[:, :PSZ].rearrange("p (a b) -> p a b", b=PW)

            # --- zero pads: engine memsets can't write f32r (ISA check), so
            # memset an F32 scratch and cast-copy (rounds to f32r). ---
            zs = xlin_pool.tile([128, 96], F32, name="zs")
            nc.vector.memset(zs[:, :], 0.0)
            # H0 top padded row 0 (incl col 0)
            nc.vector.tensor_copy(xpad[0:IC, 0:PW], zs[0:IC, 0:PW])
            # H0 bottom padded row 65 + slack (ky=2 view tail)
            nc.vector.tensor_copy(
                xpad[0:IC, (PH - 1) * PW:ALLOC],
                zs[0:IC, 0:ALLOC - (PH - 1) * PW])
            # shared zero column 0 of every padded row, both halves
            nc.vector.tensor_copy(
                xr[:, :, 0:1],
                zs[:, 0:PH].rearrange("p (a b) -> p a b", b=1))

            # --- weights ---
            nc.sync.dma_start(wsb, wt.bitcast(F32R))

            # --- x pieces: small leading pieces for an early matmul start,
            # interleaved H0/H1 so the halves become ready in lockstep ---
            PIECES = [(0, 8), (8, 8), (16, 16), (32, 16), (48, 16)]
            for r0, nr in PIECES:
                cs = slice(r0 * W, (r0 + nr) * W)
                nc.sync.dma_start(xlin[0:IC, cs], x.bitcast(F32R)[:, cs])
                nc.sync.dma_start(xlin[IC:128, cs], x.bitcast(F32R)[:, cs])

            # --- pad-copies in 8-row sub-pieces (= 1 output chunk): each
            # completion unlocks just a few matmuls, which keeps the PE
            # dispatch queue shallow (cost model rewards this with the
            # full-rate p-state).
            # H0: image row r -> padded row r+1 (Pool engine)
            # H1: image row r -> padded row r   (Activation engine)
            for j in range(H // RPC):
                src = xlin[:, j * RPC * W:(j + 1) * RPC * W].rearrange(
                    "p (a b) -> p a b", b=W)
                r0 = j * RPC
                nc.gpsimd.tensor_copy(
                    xr[0:IC, 1 + r0:1 + r0 + RPC, 1:PW], src[0:IC])
                nc.scalar.copy(
                    xr[IC:128, r0:r0 + RPC, 1:PW], src[IC:128])

            # --- conv: chunk c accumulates its 6 matmuls into PSUM bank c ---
            wr = wsb
            xrr = xpad

            # PE warmup: junk matmuls on the weight tile while x stages.
            # They keep the tensor engine's busy-streak alive so the real
            # matmuls are charged the full-rate p-state, and their output
            # (PSUM bank of chunk 0) is overwritten by its start=True matmul.
            wup = psum_pool.tile([OC, CHUNK], F32, name="ps")
            for i in range(6):
                nc.tensor.matmul(
                    wup[:, 0:384], wr[0:IC, 0:OC], wr[0:IC, :],
                    start=True, stop=(i == 5), skip_group_check=True)

            pss = []
            for c in range(NCH):
                ps = psum_pool.tile([OC, CHUNK], F32, name="ps")
                pss.append(ps)
                # fused ky=0+1 (contraction 128)
                for kx in range(K):
                    o = (c * RPC) * PW + kx
                    rhs = xrr[:, o:o + RPC * PW].rearrange(
                        "p (a b) -> p a b", b=PW)[:, :, :W]
                    nc.tensor.matmul(
                        ps[:, :], wr[:, kx * OC:(kx + 1) * OC], rhs,
                        start=(kx == 0), stop=False, skip_group_check=True)
                # ky=2 singles (contraction 64, H0 only)
                for kx in range(K):
                    o = (c * RPC + 2) * PW + kx
                    rhs = xrr[0:IC, o:o + RPC * PW].rearrange(
                        "p (a b) -> p a b", b=PW)[:, :, :W]
                    nc.tensor.matmul(
                        ps[:, :], wr[0:IC, (K + kx) * OC:(K + kx + 1) * OC],
                        rhs, start=False, stop=(kx == K - 1),
                        skip_group_check=True)

                # drain: chunk pairs early on, single chunks at the end
                # (shorter critical-path tail after the last matmul)
                if c in (1, 3, 5):
                    osb = osb_pool.tile([OC, 2 * CHUNK], F32, name="osb")
                    nc.vector.tensor_copy(osb[:, 0:CHUNK], pss[c - 1][:, :])
                    nc.vector.tensor_copy(osb[:, CHUNK:2 * CHUNK], ps[:, :])
                    nc.sync.dma_start(
                        y[:, (c - 1) * CHUNK:(c + 1) * CHUNK], osb[:, :])
                elif c in (6, 7):
                    osbs = osb_pool.tile([OC, CHUNK], F32, name="osbs")
                    nc.vector.tensor_copy(osbs[:, :], ps[:, :])
                    nc.sync.dma_start(
                        y[:, c * CHUNK:(c + 1) * CHUNK], osbs[:, :])

    nc.compile()
    return nc


_NC_CACHE: dict[str, bacc.Bacc] = {}
MODE = "v2"


def kernel(x: np.ndarray, Wt: np.ndarray) -> np.ndarray:
    assert x.shape == (8, IC, H, W) and Wt.shape == (OC, IC, K, K)
    if MODE not in _NC_CACHE:
        _NC_CACHE[MODE] = _build()
    nc = _NC_CACHE[MODE]

    Wf = Wt.astype(np.float32)
    wt_t = np.zeros((128, 2 * K * OC), dtype=np.float32)
    # [O,I,kx] -> [I,kx,O] -> [64, 192]
    wt_t[0:IC, 0:192] = Wf[:, :, 0, :].transpose(1, 2, 0).reshape(IC, 192)
    wt_t[IC:128, 0:192] = Wf[:, :, 1, :].transpose(1, 2, 0).reshape(IC, 192)
    wt_t[0:IC, 192:384] = Wf[:, :, 2, :].transpose(1, 2, 0).reshape(IC, 192)

    in_maps = [
        {
            "x": np.ascontiguousarray(
                x[b].reshape(IC, HWPIX), dtype=np.float32),
            "wt": wt_t,
        }
        for b in range(8)
    ]
    global _last_in_maps
    _last_in_maps = in_maps
    res = run_bass_kernel_spmd(nc, in_maps, core_ids=list(range(8)))
    out = np.stack([r["y"].reshape(OC, H, W) for r in res.results])
    return out.astype(np.float32)


_last_in_maps: list[dict[str, np.ndarray]] = []
